# revision 6
# baseline (speedup 1.0000x reference)
"""Deformable conv (DCNv2) + BN + ReLU Trainium2 Bass kernel.

Sharding: 8 cores = (2 batches) x (4 H-strips of 32 rows). Each core:
  1. 3x3 offset/mask conv via PE matmuls (pixel-major output, bias via
     ones-channel trick).  Chunked (first 4 tiles, then the rest) so the
     gather stream starts early.
  2. Bilinear sample positions -> per-pixel patch indices + 4 cell weights
     (DVE, chunked like phase 1).
  3. Gather 2x2x64ch patches from a precomputed patch buffer in DRAM via
     indirect DMA (1KB/descriptor, 9 per tile).
  4. Weight cells (DVE), fold, PE-transpose to channel-major, main conv via
     PE matmuls accumulating in PSUM.
  5. BN stats partial sums -> AllGather across 8 cores + local combine ->
     scale/shift + ReLU in 4 chunks overlapped with the output DMA.

Host side prepares per-core staged inputs (slab with halo+padding+ones row,
patch buffer, constants) and reassembles the output.
"""

import os
import numpy as np
import concourse.bass as bass
import concourse.mybir as mybir
import concourse.tile as tile
from concourse.bass_utils import run_bass_kernel_spmd
from contextlib import ExitStack

F32 = mybir.dt.float32
I32 = mybir.dt.int32

B, C, O, H, W = 2, 64, 64, 128, 128
NCORES = 8
RPC = H // 4            # rows per core (4 strips per batch)
NPIX_TOT = B * H * W    # 32768 (BN denominator)
BN_EPS = 1e-5


def _sap(ap, off_elems, dims):
    """AP with same tensor/partition dim, custom free dims."""
    return bass.AP(ap.tensor, ap.offset + off_elems, [ap.ap[0]] + dims)


def fix_multiwait(nc):
    """This env's walrus allows only ONE sem wait per instruction; split
    extras into single-wait drains on the same engine immediately before."""
    for f in nc.m.functions:
        for blk in f.blocks:
            i = 0
            while i < len(blk.instructions):
                ins = blk.instructions[i]
                si = ins.sync_info
                if si is not None and si.on_wait and len(si.on_wait) > 1:
                    waits = list(si.on_wait)
                    si.on_wait = [waits[-1]]
                    for j, w in enumerate(waits[:-1]):
                        d2 = mybir.InstDrain(
                            name=f"{ins.name}-wsplit{j}", ins=[], outs=[],
                            engine=ins.engine,
                        )
                        d2.sync_info = mybir.SyncInfo(on_wait=[w], on_update=[])
                        blk.instructions.insert(i, d2)
                        i += 1
                i += 1


def build_nc(T=RPC, collective=True, fixup=True):
    """Build the per-core Bass module. T = number of row-tiles (32 normally)."""
    nc = bass.Bass()
    K9 = 9
    ELEM = 256             # 2x2 patch x 64 ch, f32
    CH0 = 2                # tiles in the first phase-1/2 chunk

    # ---- per-core external inputs (host-staged) ----
    xslab = nc.dram_tensor("xslab", [C + 1, T + 2, W + 2], F32, kind="ExternalInput")
    pbc = nc.dram_tensor("pbc", [4 * 4096, ELEM], F32, kind="ExternalInput")
    cy = nc.dram_tensor("cy", [128, T, K9], F32, kind="ExternalInput")
    cx = nc.dram_tensor("cx", [128, T, K9], F32, kind="ExternalInput")
    wcat = nc.dram_tensor("wcat", [C + 1, K9, 27], F32, kind="ExternalInput")
    w2 = nc.dram_tensor("w2", [C, K9, O], F32, kind="ExternalInput")
    ident = nc.dram_tensor("ident", [128, 128], F32, kind="ExternalInput")
    gb = nc.dram_tensor("gb", [O, 2], F32, kind="ExternalInput")
    outd = nc.dram_tensor("outn", [O, T, W], F32, kind="ExternalOutput")

    with tile.TileContext(nc) as tc:
        with ExitStack() as ctx:
            cpool = ctx.enter_context(tc.tile_pool(name="const", bufs=1))
            ppool = ctx.enter_context(tc.tile_pool(name="persist", bufs=1))
            wpool = ctx.enter_context(tc.tile_pool(name="wtmp", bufs=1))
            gpool = ctx.enter_context(tc.tile_pool(name="gath", bufs=3))
            mpool = ctx.enter_context(tc.tile_pool(name="mac", bufs=2))
            psA = ctx.enter_context(tc.tile_pool(name="psA", bufs=2, space="PSUM"))
            psB = ctx.enter_context(tc.tile_pool(name="psB", bufs=2, space="PSUM"))
            dpool = ctx.enter_context(tc.tile_pool(name="dram", bufs=1, space="DRAM"))

            TT = nc.vector.tensor_tensor
            TS = nc.vector.tensor_scalar
            STT = nc.vector.scalar_tensor_tensor
            AL = mybir.AluOpType

            # ---- prologue loads ----
            wc = cpool.tile([C + 1, K9, 27], F32, tag="wc")
            nc.sync.dma_start(wc[:], wcat[:])
            xs = cpool.tile([C + 1, T + 2, W + 2], F32, tag="xs")
            nc.sync.dma_start(xs[:], xslab[:])
            cys = cpool.tile([128, T, K9], F32, tag="cys")
            nc.sync.dma_start(cys[:], cy[:])
            cxs = cpool.tile([128, T, K9], F32, tag="cxs")
            nc.sync.dma_start(cxs[:], cx[:])
            w2s = cpool.tile([C, K9, O], F32, tag="w2s")
            nc.sync.dma_start(w2s[:], w2[:])
            idt = cpool.tile([128, 128], F32, tag="idt")
            nc.sync.dma_start(idt[:], ident[:])
            gbs = cpool.tile([O, 2], F32, tag="gbs")
            nc.sync.dma_start(gbs[:], gb[:])
            epst = cpool.tile([128, 1], F32, tag="epst")
            nc.vector.memset(epst[:], BN_EPS)

            # ---- persistent tiles ----
            OFF = ppool.tile([128, T, 27], F32, tag="OFF")
            MK = ppool.tile([128, T, K9], F32, tag="MK")
            W4 = ppool.tile([128, T, K9, 4], F32, tag="W4")
            IDX = ppool.tile([128, T, K9], I32, tag="IDX")
            ST1 = ppool.tile([O, T], F32, tag="ST1")
            ST2 = ppool.tile([O, T], F32, tag="ST2")
            OPRE = ppool.tile([O, T, W], F32, tag="OPRE")
            ON = ppool.tile([O, T, W], F32, tag="ON")

            def phase1(t0, t1):
                """offset/mask conv for tiles [t0, t1) (pixel-major out)."""
                for t in range(t0, t1):
                    pso = psA.tile([128, 27], F32, tag="big")
                    for k in range(K9):
                        ky, kx = k // 3, k % 3
                        lhsT = _sap(xs[:], (t + ky) * (W + 2) + kx, [[1, 128]])
                        rhs = _sap(wc[:], k * 27, [[1, 27]])
                        nc.tensor.matmul(pso[:], lhsT, rhs,
                                         start=(k == 0), stop=(k == K9 - 1))
                    nc.vector.tensor_copy(OFF[:, t, :], pso[:])
                # batched sigmoid for the chunk
                nc.scalar.activation(MK[:, t0:t1, :], OFF[:, t0:t1, 18:27],
                                     mybir.ActivationFunctionType.Sigmoid)

            def phase2(t0, t1):
                """sample coords -> cell weights + gather indices, tiles [t0,t1)."""
                Tc = t1 - t0
                dd = lambda tag: wpool.tile([128, Tc, K9], F32, tag=f"{tag}_{t0}", name=f"{tag}_{t0}")

                def floorfix(dst_fl, src, tag):
                    ri = wpool.tile([128, Tc, K9], I32, tag=f"ri_{tag}_{t0}", name=f"ri_{tag}_{t0}")
                    nc.vector.tensor_copy(ri[:], src)
                    rf = dd(f"rf_{tag}")
                    nc.vector.tensor_copy(rf[:], ri[:])
                    g = dd(f"g_{tag}")
                    TT(g[:], rf[:], src, AL.is_gt)
                    TT(dst_fl, rf[:], g[:], AL.subtract)

                offy = _sap(OFF[:], t0 * 27, [[27, Tc], [2, K9]])
                offx = _sap(OFF[:], t0 * 27 + 1, [[27, Tc], [2, K9]])
                mkc = MK[:, t0:t1, :]
                py = dd("py")
                TT(py[:], offy, cys[:, t0:t1, :], AL.add)
                px = dd("px")
                TT(px[:], offx, cxs[:, t0:t1, :], AL.add)

                Y0 = dd("Y0")
                floorfix(Y0[:], py[:], "y")
                X0 = dd("X0")
                floorfix(X0[:], px[:], "x")
                WY = dd("WY")
                TT(WY[:], py[:], Y0[:], AL.subtract)
                WX = dd("WX")
                TT(WX[:], px[:], X0[:], AL.subtract)

                def vpair(F0, wfrac, tag):
                    a1 = dd(f"a1_{tag}")
                    TS(a1[:], F0, 0.0, None, AL.is_ge)
                    a2 = dd(f"a2_{tag}")
                    TS(a2[:], F0, 126.0, None, AL.is_le)
                    A = dd(f"A_{tag}")
                    TT(A[:], a1[:], a2[:], AL.mult)
                    Bq = dd(f"B_{tag}")
                    TS(Bq[:], F0, -1.0, None, AL.is_equal)
                    Cq = dd(f"C_{tag}")
                    TS(Cq[:], F0, 127.0, None, AL.is_equal)
                    om = dd(f"om_{tag}")
                    TS(om[:], wfrac, -1.0, 1.0, AL.mult, AL.add)
                    u1 = dd(f"u1_{tag}")
                    TT(u1[:], om[:], A[:], AL.mult)
                    u2 = dd(f"u2_{tag}")
                    TT(u2[:], wfrac, Bq[:], AL.mult)
                    V0 = dd(f"V0_{tag}")
                    TT(V0[:], u1[:], u2[:], AL.add)
                    u3 = dd(f"u3_{tag}")
                    TT(u3[:], wfrac, A[:], AL.mult)
                    u4 = dd(f"u4_{tag}")
                    TT(u4[:], om[:], Cq[:], AL.mult)
                    V1 = dd(f"V1_{tag}")
                    TT(V1[:], u3[:], u4[:], AL.add)
                    return V0, V1

                V0, V1 = vpair(Y0[:], WY[:], "vy")
                U0, U1 = vpair(X0[:], WX[:], "ux")
                TT(V0[:], V0[:], mkc, AL.mult)
                TT(V1[:], V1[:], mkc, AL.mult)

                def clampfloor2(F0, tag):
                    c1 = dd(f"c1_{tag}")
                    TS(c1[:], F0, 0.0, None, AL.max)
                    FB = dd(f"FB_{tag}")
                    TS(FB[:], c1[:], 126.0, None, AL.min)
                    h = dd(f"h_{tag}")
                    TS(h[:], FB[:], 0.5, None, AL.mult)
                    Iq = dd(f"I_{tag}")
                    floorfix(Iq[:], h[:], f"cf_{tag}")
                    return FB, Iq

                YB, IY = clampfloor2(Y0[:], "yb")
                XB, IX = clampfloor2(X0[:], "xb")
                PP = dd("PP")
                STT(PP[:], IY[:], -2.0, YB[:], AL.mult, AL.add)
                QQ = dd("QQ")
                STT(QQ[:], IX[:], -2.0, XB[:], AL.mult, AL.add)
                t5 = dd("t5")
                STT(t5[:], PP[:], 2.0, QQ[:], AL.mult, AL.add)
                t6 = dd("t6")
                STT(t6[:], IY[:], 64.0, IX[:], AL.mult, AL.add)
                idxf = dd("idxf")
                STT(idxf[:], t5[:], 4096.0, t6[:], AL.mult, AL.add)
                nc.vector.tensor_copy(IDX[:, t0:t1, :], idxf[:])

                w4base = t0 * K9 * 4
                TT(_sap(W4[:], w4base + 0, [[K9 * 4, Tc], [4, K9]]), V0[:], U0[:], AL.mult)
                TT(_sap(W4[:], w4base + 1, [[K9 * 4, Tc], [4, K9]]), V0[:], U1[:], AL.mult)
                TT(_sap(W4[:], w4base + 2, [[K9 * 4, Tc], [4, K9]]), V1[:], U0[:], AL.mult)
                TT(_sap(W4[:], w4base + 3, [[K9 * 4, Tc], [4, K9]]), V1[:], U1[:], AL.mult)

            # chunked phases 1+2: small head chunk, then the rest
            phase1(0, CH0)
            phase2(0, CH0)
            phase1(CH0, T)
            for a in range(CH0, T, 10):
                phase2(a, min(a + 10, T))

            # ---- phase 4: gather + weight + transpose + conv ----
            for t in range(T):
                G = gpool.tile([128, K9, 4, C], F32, tag="g")
                for k in range(K9):
                    nc.gpsimd.indirect_dma_start(
                        G[:, k, :, :].rearrange("p a c -> p (a c)"), None, pbc[:],
                        bass.IndirectOffsetOnAxis(ap=IDX[:, t, k : k + 1], axis=0),
                    )
                prod = mpool.tile([128, K9, 4, C], F32, tag="prod")
                w4b = _sap(W4[:], t * K9 * 4, [[4, K9], [1, 4], [0, C]])
                TT(prod[:], G[:], w4b, AL.mult)
                cadd = mpool.tile([128, K9, 2, C], F32, tag="cadd")
                nc.vector.tensor_tensor(cadd[:], prod[:, :, 0:2, :], prod[:, :, 2:4, :], AL.add)
                val = mpool.tile([128, K9, C], F32, tag="val")
                nc.vector.tensor_tensor(val[:], cadd[:, :, 0, :], cadd[:, :, 1, :], AL.add)

                tp = psA.tile([C, K9, 128], F32, tag="big")
                for k in range(K9):
                    nc.tensor.matmul(tp[:, k, :], val[:, k, :], idt[:],
                                     is_transpose=True, start=True, stop=True)
                vch = mpool.tile([C, K9, 128], F32, tag="vch")
                nc.scalar.copy(vch[:], tp[:])

                po = psB.tile([O, 128], F32, tag="po")
                for k in range(K9):
                    nc.tensor.matmul(po[:], w2s[:, k, :], vch[:, k, :],
                                     start=(k == 0), stop=(k == K9 - 1))
                nc.scalar.copy(OPRE[:, t, :], po[:])
                nc.vector.tensor_reduce(ST1[:, t : t + 1], OPRE[:, t, :],
                                        mybir.AxisListType.X, AL.add)
                sq = mpool.tile([O, 128], F32, tag="sq")
                TT(sq[:], OPRE[:, t, :], OPRE[:, t, :], AL.mult)
                nc.vector.tensor_reduce(ST2[:, t : t + 1], sq[:],
                                        mybir.AxisListType.X, AL.add)

            # ---- phase 5: BN + ReLU ----
            s1 = ppool.tile([O, 2], F32, tag="s1")
            nc.vector.tensor_reduce(s1[:, 0:1], ST1[:], mybir.AxisListType.X, AL.add)
            nc.vector.tensor_reduce(s1[:, 1:2], ST2[:], mybir.AxisListType.X, AL.add)
            if collective:
                cin = dpool.tile([O, 2], F32, tag="cin")
                cout = dpool.tile([NCORES * O, 2], F32, tag="cout")
                nc.sync.dma_start(cin[:], s1[:])
                nc.gpsimd.collective_compute(
                    "AllGather", AL.bypass,
                    replica_groups=[list(range(NCORES))],
                    ins=[cin.opt()], outs=[cout.opt()],
                )
                sg8 = ppool.tile([O, 2, NCORES], F32, tag="sg8")
                nc.sync.dma_start(
                    sg8[:],
                    bass.AP(cout[:].tensor, 0, [[2, O], [1, 2], [2 * O, NCORES]]),
                )
                sg = ppool.tile([O, 2], F32, tag="sg")
                nc.vector.tensor_reduce(sg[:], sg8[:], mybir.AxisListType.X, AL.add)
                denom = float(NPIX_TOT)
            else:
                sg = s1
                denom = float(T * W)

            mean = ppool.tile([O, 1], F32, tag="mean")
            TS(mean[:], sg[:, 0:1], 1.0 / denom, None, AL.mult)
            ex2 = ppool.tile([O, 1], F32, tag="ex2")
            TS(ex2[:], sg[:, 1:2], 1.0 / denom, None, AL.mult)
            m2 = ppool.tile([O, 1], F32, tag="m2")
            TT(m2[:], mean[:], mean[:], AL.mult)
            var = ppool.tile([O, 1], F32, tag="var")
            TT(var[:], ex2[:], m2[:], AL.subtract)
            stdt = ppool.tile([O, 1], F32, tag="stdt")
            nc.scalar.activation(stdt[:], var[:],
                                 mybir.ActivationFunctionType.Sqrt,
                                 bias=epst[0:O, :])
            rstd = ppool.tile([O, 1], F32, tag="rstd")
            nc.vector.reciprocal(rstd[:], stdt[:])
            scl = ppool.tile([O, 1], F32, tag="scl")
            TT(scl[:], gbs[:, 0:1], rstd[:], AL.mult)
            msc = ppool.tile([O, 1], F32, tag="msc")
            TT(msc[:], mean[:], scl[:], AL.mult)
            sh = ppool.tile([O, 1], F32, tag="sh")
            TT(sh[:], gbs[:, 1:2], msc[:], AL.subtract)

            # fused scale/shift + ReLU in 4 chunks, each followed by its DMA
            NCH = 4
            step = T // NCH
            for i in range(NCH):
                a, b = i * step, (i + 1) * step
                nc.scalar.activation(ON[:, a:b, :], OPRE[:, a:b, :],
                                     mybir.ActivationFunctionType.Relu,
                                     bias=sh[:], scale=scl[:])
                nc.sync.dma_start(outd[:, a:b, :], ON[:, a:b, :])

    if fixup:
        fix_multiwait(nc)
    return nc


# ---------------- host-side preparation ----------------

def _host_prep(x, conv_w, off_w, off_b, mask_w, mask_b, gamma, beta, T=RPC):
    """Build the 8 per-core input maps."""
    x = np.asarray(x, np.float32)
    K9 = 9

    # patch buffer per batch: PB[b, 2p+q, i*64+j, :] = 2x2 patch at
    # rows (2i+p, 2i+p+1), cols (2j+q, 2j+q+1), channels-last, flattened.
    xcl = np.transpose(x, (0, 2, 3, 1))  # [B, H, W, C]
    xpad = np.zeros((B, H + 2, W + 2, C), np.float32)
    xpad[:, :H, :W] = xcl
    PB = np.zeros((B, 4, 4096, 256), np.float32)
    for p in range(2):
        for q in range(2):
            sub = xpad[:, p : p + 129, q : q + 129, :]
            r0 = sub[:, 0:128:2, 0:128:2, :]
            r1 = sub[:, 0:128:2, 1:129:2, :]
            r2 = sub[:, 1:129:2, 0:128:2, :]
            r3 = sub[:, 1:129:2, 1:129:2, :]
            patch = np.stack([r0, r1, r2, r3], axis=3)  # [B, 64, 64, 4, C]
            PB[:, 2 * p + q] = patch.reshape(B, 4096, 256)

    wfull = np.concatenate([off_w, mask_w], axis=0)  # [27, C, 3, 3]
    wcat = np.zeros((C + 1, K9, 27), np.float32)
    wcat[:C] = np.transpose(wfull.reshape(27, C, K9), (1, 2, 0))
    bias = np.concatenate([off_b, mask_b]).astype(np.float32)  # [27]
    wcat[C, 0, :] = bias

    w2 = np.transpose(conv_w.reshape(O, C, K9), (1, 2, 0)).astype(np.float32)  # [C, 9, O]
    ident = np.eye(128, dtype=np.float32)
    gb = np.stack([np.asarray(gamma, np.float32), np.asarray(beta, np.float32)], axis=1)

    ky = np.repeat(np.arange(3), 3).astype(np.float32)
    kx = np.tile(np.arange(3), 3).astype(np.float32)
    gx = np.arange(128, dtype=np.float32)

    in_maps = []
    for core in range(NCORES):
        b, strip = divmod(core, 4)
        r0 = strip * RPC
        xslab = np.zeros((C + 1, T + 2, W + 2), np.float32)
        lo, hi = r0 - 1, r0 + T + 1
        glo, ghi = max(lo, 0), min(hi, H)
        xslab[:C, (glo - lo) : (ghi - lo), 1 : W + 1] = x[b, :, glo:ghi, :]
        xslab[C] = 1.0
        cy = (r0 + np.arange(T)[None, :, None] + (ky - 1.0)[None, None, :]
              + np.zeros((128, 1, 1))).astype(np.float32)
        cx = (gx[:, None, None] + (kx - 1.0)[None, None, :]
              + np.zeros((1, T, 1))).astype(np.float32)
        in_maps.append({
            "xslab": xslab, "pbc": PB[b].reshape(4 * 4096, 256),
            "cy": cy, "cx": cx, "wcat": wcat, "w2": w2,
            "ident": ident, "gb": gb,
        })
    return in_maps


_NC_CACHE = {}


def kernel(x, conv_w, off_w, off_b, mask_w, mask_b, gamma, beta):
    if "nc" not in _NC_CACHE:
        _NC_CACHE["nc"] = build_nc()
    nc = _NC_CACHE["nc"]
    in_maps = _host_prep(x, conv_w, off_w, off_b, mask_w, mask_b, gamma, beta)
    res = run_bass_kernel_spmd(nc, in_maps, core_ids=list(range(NCORES)))
    out = np.zeros((B, O, H, W), np.float32)
    for core in range(NCORES):
        b, strip = divmod(core, 4)
        r0 = strip * RPC
        out[b, :, r0 : r0 + RPC, :] = res.results[core]["outn"]
    return out
